# Initial kernel scaffold
#
"""Trainium2 Bass kernel for the AttentionBlock problem.

Problem (hardcoded): x (16, 512, 32, 32) fp32; GroupNorm(32 groups) ->
1x1-conv QKV (1536x512) -> 4-head attention over 1024 tokens, head dim 128
-> 1x1-conv proj (512x512) -> residual add.

Sharding: data-parallel over batch, 2 batches per core on 8 cores; params
replicated. Weights are pre-transposed (and cast to bf16) on the host so
every matmul operand is consumed in its natural [contract-dim-on-partitions,
free] layout.

Per-core dataflow (per batch):
  - GroupNorm: per-channel bn_stats/bn_aggr in fp32, group (16-channel)
    reduction and broadcast-back via tiny fp32r matmuls against constant
    group-membership masks (host inputs).
  - QKV in bf16: q, k produced as [d, n] (heads = 128-row chunks); v
    produced directly transposed as [n, d] by swapping the matmul operands.
  - Attention per head (bf16 matmuls, fp32 PSUM accumulation):
    T = K^T Q (keys on partitions); P^T = exp(T/sqrt(hd)) — the s=0 query
    half on the scalar engine (true exp), the s=1 half on the vector engine
    via the Schraudolph bit trick (scale+shift into the bf16 exponent field,
    written through an int16 bitcast view).  No max subtraction needed:
    |T/sqrt(hd)| < ~7.  O^T accumulated over key chunks; softmax
    denominators via matmuls with an all-ones [128,128] stationary matrix,
    which lands the sums in PSUM replicated across partitions: reciprocal +
    multiply finish softmax with no broadcast step.  Each query column uses
    one exp method for all its keys, so per-query normalization stays exact.
  - Proj (bf16) + residual (fp32), streamed back to DRAM.
  - The two batches' phases are interleaved (batch-1 norm+qkv emitted
    before batch-0 proj) to hide the attention tail.
"""

import math

import numpy as np
import ml_dtypes

import concourse.mybir as mybir
import concourse.tile as tile
from concourse import bacc
from concourse.bass_utils import run_bass_kernel_spmd

# Problem constants
B, C, N = 16, 512, 1024          # batch, channels, tokens (32*32)
HEADS, HD = 4, 128               # heads, head dim
GROUPS, GS = 32, 16              # norm groups, channels per group
EPS = 1e-5
N_CORES = 8
BL = B // N_CORES                # batches per core
CC = C // 128                    # channel chunks of 128
SCALE = 1.0 / math.sqrt(HD)

F32 = mybir.dt.float32
F32R = mybir.dt.float32r
BF16 = mybir.dt.bfloat16
FP8 = mybir.dt.float8e4
I16 = mybir.dt.int16
BF16_NP = ml_dtypes.bfloat16
FP8_NP = ml_dtypes.float8_e4m3

# Softmax exp shift: P = exp(t - EXPSHIFT) keeps P in fp8e4 range (TRN max
# 240); the shift cancels exactly in the normalization.
EXPSHIFT = 4.0
# Schraudolph fast-exp into bf16 bits: bits16 = round(t*128/ln2 + b) where
# t = SCALE*s - EXPSHIFT.  b = 127*128 - 7.4 centers the mantissa-interp
# error.
EXPA = SCALE * (128.0 / math.log(2.0))
EXPB = 127.0 * 128.0 - 7.4 - EXPSHIFT * (128.0 / math.log(2.0))

DR = mybir.MatmulPerfMode.DoubleRow


def _mm(nc, out, lhsT, rhs, start=True, stop=True, perf_mode=None):
    nc.tensor.matmul(out, lhsT, rhs, start=start, stop=stop,
                     perf_mode=perf_mode)


def build(reps=1):
    nc = bacc.Bacc("TRN2", target_bir_lowering=False, debug=False)

    x_d = nc.dram_tensor("x", [BL, C, N], F32, kind="ExternalInput").ap()
    nw_d = nc.dram_tensor("norm_w", [C], F32, kind="ExternalInput").ap()
    nb_d = nc.dram_tensor("norm_b", [C], F32, kind="ExternalInput").ap()
    # DoubleRow pair layout: [pair, partition, j, cols] with contraction
    # channel c = (2*pair + j)*128 + partition
    wq_d = nc.dram_tensor("qkv_w8", [2, 128, 2, 3 * C], FP8,
                          kind="ExternalInput").ap()
    qb_d = nc.dram_tensor("qkv_b", [2 * C], F32, kind="ExternalInput").ap()
    wp_d = nc.dram_tensor("proj_w8", [2, 128, 2, C], FP8,
                          kind="ExternalInput").ap()
    pb_d = nc.dram_tensor("proj_b", [C], F32, kind="ExternalInput").ap()
    gm_d = nc.dram_tensor("gmask", [CC, 128, GROUPS], F32R,
                          kind="ExternalInput").ap()
    gmT_d = nc.dram_tensor("gmaskT", [CC, GROUPS, 128], F32R,
                           kind="ExternalInput").ap()
    out_d = nc.dram_tensor("out", [BL, C, N], F32, kind="ExternalOutput").ap()

    with tile.TileContext(nc) as tc:
        with (
            nc.allow_low_precision(reason="bf16 tiles feeding bf16 matmuls"),
            tc.tile_pool(name="const", bufs=1) as const,
            tc.tile_pool(name="xp", bufs=2) as xp,
            tc.tile_pool(name="xnp", bufs=2) as xnp,
            tc.tile_pool(name="qkp", bufs=1) as qkp,
            tc.tile_pool(name="vtp", bufs=1) as vtp,
            tc.tile_pool(name="ptp", bufs=6) as ptp,
            tc.tile_pool(name="schp", bufs=2) as schp,
            tc.tile_pool(name="xpbp", bufs=2) as xpbp,
            tc.tile_pool(name="ocp", bufs=1) as ocp,
            tc.tile_pool(name="smallp", bufs=4) as smallp,
            tc.tile_pool(name="rbp", bufs=2) as rbp,
            tc.tile_pool(name="yp", bufs=4) as yp,
            tc.tile_pool(name="ps_work", bufs=4, space="PSUM") as ps_work,
            tc.tile_pool(name="ps_o", bufs=2, space="PSUM") as ps_o,
            tc.tile_pool(name="ps_sum", bufs=2, space="PSUM") as ps_sum,
        ):
            # ---- constants / weights (loaded once) ----
            # ones first: it feeds the PE warm-up matmuls and the softmax
            # denominator (DoubleRow) matmuls
            ones8 = const.tile([128, 2, 128], FP8, name="ones8")
            nc.vector.memset(ones8, 1.0)
            for wi in range(2):
                wu_ps = ps_work.tile([128, 128], F32, tag="w",
                                     name=f"wu{wi}")
                for wj in range(6):
                    _mm(nc, wu_ps, ones8[:, 0, :], ones8[:, 0, :],
                        start=(wj == 0), stop=(wj == 5))

            # small consts + masks next: they gate the first stats matmuls
            w_sb = const.tile([128, CC], F32, name="w_sb")
            nc.scalar.dma_start(
                out=w_sb, in_=nw_d.rearrange("(cc p) -> p cc", p=128))
            b_sb = const.tile([128, CC], F32, name="b_sb")
            nc.scalar.dma_start(
                out=b_sb, in_=nb_d.rearrange("(cc p) -> p cc", p=128))
            qb_sb = const.tile([128, 8], F32, name="qb_sb")
            nc.scalar.dma_start(
                out=qb_sb, in_=qb_d.rearrange("(oc p) -> p oc", p=128)
            )
            pb_sb = const.tile([128, CC], F32, name="pb_sb")
            nc.scalar.dma_start(
                out=pb_sb, in_=pb_d.rearrange("(cc p) -> p cc", p=128))

            eps_t = const.tile([GROUPS, 1], F32, name="eps_t")
            nc.vector.memset(eps_t, EPS)
            nshift_t = const.tile([128, 1], F32, name="nshift_t")
            nc.vector.memset(nshift_t, -EXPSHIFT)

            gm = []
            gmT = []
            for cc in range(CC):
                t = const.tile([128, GROUPS], F32R, name=f"gm{cc}")
                nc.scalar.dma_start(out=t, in_=gm_d[cc])
                tT = const.tile([GROUPS, 128], F32R, name=f"gmT{cc}")
                nc.scalar.dma_start(out=tT, in_=gmT_d[cc])
                gm.append(t)
                gmT.append(tT)

            wq_sb = []
            for pr in range(2):
                t = const.tile([128, 2, 3 * C], FP8, name=f"wq{pr}")
                nc.sync.dma_start(out=t, in_=wq_d[pr])
                wq_sb.append(t)
            wp_sb = []
            for pr in range(2):
                t = const.tile([128, 2, C], FP8, name=f"wp{pr}")
                nc.scalar.dma_start(out=t, in_=wp_d[pr])
                wp_sb.append(t)

            # ---- per batch pipeline (phases interleaved across batches) ----
            def norm(b):
                x_t = xp.tile([128, CC, N], F32, tag="x", name=f"x_t{b}")
                xr = x_d[b].rearrange("(cc p) n -> p cc n", p=128)
                for cc in range(CC):
                    eng = nc.gpsimd if cc % 2 == 0 else nc.scalar
                    eng.dma_start(out=x_t[:, cc, :], in_=xr[:, cc, :])

                # group stats via per-channel bn_stats/bn_aggr
                cols = smallp.tile([128, CC, 2], F32R, tag="mv",
                                   name=f"cols{b}")
                for cc in range(CC):
                    stats = smallp.tile([128, 2, 6], F32, tag="stats",
                                        name=f"stats{b}_{cc}")
                    for s in range(2):
                        nc.vector.bn_stats(
                            out=stats[:, s, :],
                            in_=x_t[:, cc, s * 512:(s + 1) * 512],
                        )
                    mv_f = smallp.tile([128, 2], F32, tag="mvf",
                                       name=f"mvf{b}_{cc}")
                    nc.vector.bn_aggr(out=mv_f, in_=stats)
                    # mv[:,1] := E[x^2] = var + mean^2
                    msq = smallp.tile([128, 1], F32, tag="msq",
                                      name=f"msq{b}_{cc}")
                    nc.vector.tensor_mul(msq, mv_f[:, 0:1], mv_f[:, 0:1])
                    nc.vector.tensor_add(mv_f[:, 1:2], mv_f[:, 1:2], msq)
                    nc.vector.tensor_copy(out=cols[:, cc, :], in_=mv_f)

                gstats = ps_sum.tile([GROUPS, 2], F32, tag="s",
                                     name=f"gstats{b}")
                for cc in range(CC):
                    _mm(nc, gstats, gm[cc], cols[:, cc, :],
                        start=(cc == 0), stop=(cc == CC - 1))
                grp = smallp.tile([GROUPS, 2], F32R, tag="grp", name=f"grp{b}")
                nc.scalar.mul(out=grp, in_=gstats, mul=1.0 / GS)
                gvar = smallp.tile([GROUPS, 1], F32, tag="gvar",
                                   name=f"gvar{b}")
                nc.vector.tensor_mul(gvar, grp[:, 0:1], grp[:, 0:1])
                nc.vector.tensor_sub(gvar, grp[:, 1:2], gvar)
                nc.vector.tensor_scalar(
                    out=gvar, in0=gvar, scalar1=EPS, scalar2=None,
                    op0=mybir.AluOpType.add,
                )
                # rstd via bit-trick rsqrt + 2 Newton steps, all on DVE:
                # avoids the ACT Sqrt<->Exp table switch every iteration
                y0 = smallp.tile([GROUPS, 1], F32, tag="y0", name=f"y0{b}")
                nc.vector.tensor_scalar(
                    out=y0.bitcast(mybir.dt.int32),
                    in0=gvar.bitcast(mybir.dt.int32),
                    scalar1=1, scalar2=-1,
                    op0=mybir.AluOpType.logical_shift_right,
                    op1=mybir.AluOpType.bitwise_xor,
                )
                nc.vector.tensor_scalar(
                    out=y0.bitcast(mybir.dt.int32),
                    in0=y0.bitcast(mybir.dt.int32),
                    scalar1=0x5f3759df + 1, scalar2=None,
                    op0=mybir.AluOpType.add,
                )
                hv = smallp.tile([GROUPS, 1], F32, tag="hv", name=f"hv{b}")
                nc.vector.tensor_scalar(
                    out=hv, in0=gvar, scalar1=-0.5, scalar2=None,
                    op0=mybir.AluOpType.mult,
                )
                for it in range(2):
                    yy = smallp.tile([GROUPS, 1], F32, tag="yy",
                                     name=f"yy{b}_{it}")
                    nc.vector.tensor_mul(yy, y0, y0)
                    nc.vector.tensor_mul(yy, yy, hv)
                    nc.vector.tensor_scalar(
                        out=yy, in0=yy, scalar1=1.5, scalar2=None,
                        op0=mybir.AluOpType.add,
                    )
                    nc.vector.tensor_mul(y0, y0, yy)
                nc.vector.tensor_copy(out=grp[:, 1:2], in_=y0)

                # broadcast per-group (mean, rstd) back to channels, all
                # four chunk matmuls into one PSUM tile, then one vectorized
                # scale/bias computation for every chunk at once
                xn_t = xnp.tile([128, CC, N], FP8, tag="xn", name=f"xn{b}")
                bc = ps_sum.tile([128, CC, 2], F32, tag="s",
                                 name=f"bc{b}")
                for cc in range(CC):
                    _mm(nc, bc[:, cc, :], gmT[cc], grp)
                ab = smallp.tile([128, CC, 2], F32, tag="ab", name=f"ab{b}")
                # a = rstd*w  (strided [:,:,1] lanes)
                nc.vector.tensor_mul(ab[:, :, 1], bc[:, :, 1], w_sb)
                # b = norm_b - mean*a
                nc.vector.tensor_mul(ab[:, :, 0], bc[:, :, 0], ab[:, :, 1])
                nc.vector.tensor_sub(ab[:, :, 0], b_sb, ab[:, :, 0])
                for cc in range(CC):
                    if cc % 2 == 0:
                        nc.vector.tensor_scalar(
                            out=xn_t[:, cc, :], in0=x_t[:, cc, :],
                            scalar1=ab[:, cc, 1:2], scalar2=ab[:, cc, 0:1],
                            op0=mybir.AluOpType.mult, op1=mybir.AluOpType.add,
                        )
                    else:
                        nc.scalar.activation(
                            out=xn_t[:, cc, :], in_=x_t[:, cc, :],
                            func=mybir.ActivationFunctionType.Identity,
                            bias=ab[:, cc, 0:1], scale=ab[:, cc, 1:2],
                        )
                return x_t, xn_t

            def qkv(b, xn_t):
                # qkv via fp8 DoubleRow (contraction pairs of 128-ch chunks)
                q_t = qkp.tile([128, HEADS, N], BF16, tag="q", name=f"q{b}")
                k_t = qkp.tile([128, HEADS, N], BF16, tag="k", name=f"k{b}")
                vt_t = vtp.tile([128, 8, C], FP8, tag="vt", name=f"vt{b}")

                def v_chunk(tc_i):
                    acc = ps_work.tile([128, 512], F32, tag="w",
                                       name=f"vacc{b}_{tc_i}")
                    for pr in range(2):
                        _mm(nc, acc,
                            xn_t[:, 2 * pr:2 * pr + 2,
                                 tc_i * 128:(tc_i + 1) * 128],
                            wq_sb[pr][:, :, 2 * C:3 * C],
                            start=(pr == 0), stop=(pr == 1),
                            perf_mode=DR)
                    nc.vector.tensor_copy(out=vt_t[:, tc_i, :], in_=acc)

                # head h's q (oc=h) and k (oc=4+h) emitted adjacently so
                # attention of head 0 starts after two oc blocks, with the
                # v chunks it needs interleaved right behind
                for oc in [0, 4, 1, 5, 2, 6, 3, 7]:
                    dst = q_t if oc < 4 else k_t
                    h = oc % 4
                    accs = [ps_work.tile([128, 512], F32, tag="w",
                                         name=f"qkacc{b}_{oc}_{s}")
                            for s in range(2)]
                    for pr in range(2):
                        for s in range(2):  # consecutive mms share lhsT
                            _mm(nc, accs[s],
                                wq_sb[pr][:, :, oc * 128:(oc + 1) * 128],
                                xn_t[:, 2 * pr:2 * pr + 2,
                                     s * 512:(s + 1) * 512],
                                start=(pr == 0), stop=(pr == 1),
                                perf_mode=DR)
                    for s in range(2):  # both bias-adds on ACT: DVE is the
                        nc.scalar.activation(  # busier engine here
                            out=dst[:, h, s * 512:(s + 1) * 512],
                            in_=accs[s],
                            func=mybir.ActivationFunctionType.Identity,
                            bias=qb_sb[:, oc:oc + 1], scale=1.0,
                        )
                # v bias is folded into proj_b on the host
                # (attention(V + b) = attention(V) + b), so vt is a plain
                # PSUM evacuation
                vt_t = vtp.tile([128, 8, C], FP8, tag="vt", name=f"vt{b}")
                for tc_i in range(8):
                    acc = ps_work.tile([128, 512], F32, tag="w",
                                       name=f"vacc{b}_{tc_i}")
                    for pr in range(2):
                        _mm(nc, acc,
                            xn_t[:, 2 * pr:2 * pr + 2,
                                 tc_i * 128:(tc_i + 1) * 128],
                            wq_sb[pr][:, :, 2 * C:3 * C],
                            start=(pr == 0), stop=(pr == 1),
                            perf_mode=DR)
                    nc.vector.tensor_copy(out=vt_t[:, tc_i, :], in_=acc)

                return q_t, k_t, vt_t

            def attn(b, q_t, k_t, vt_t):
                ocat = ocp.tile([128, HEADS, N], FP8, tag="ocat",
                                name=f"ocat{b}")
                for h in range(HEADS):
                    o_ps = [ps_o.tile([128, 512], F32, tag="o",
                                      name=f"o{b}_{h}_{i}") for i in range(2)]
                    s_ps = [ps_sum.tile([128, 512], F32, tag="s",
                                        name=f"s{b}_{h}_{i}")
                            for i in range(2)]
                    # P^T stored as fp8 key-chunk pairs for DoubleRow AV
                    pt_pairs = [
                        ptp.tile([128, 2, N], FP8, tag="pt",
                                 name=f"pt{b}_{h}_{p}")
                        for p in range(4)
                    ]

                    def emit_t(mc):
                        pt = pt_pairs[mc // 2]
                        j = mc % 2
                        for s in range(2):
                            tps = ps_work.tile([128, 512], F32, tag="w",
                                               name=f"t{b}_{h}_{mc}_{s}")
                            _mm(nc, tps,
                                k_t[:, h, mc * 128:(mc + 1) * 128],
                                q_t[:, h, s * 512:(s + 1) * 512])
                            if (mc * 2 + s) % 16 < 11:
                                # true exp on the scalar engine (fp8 out)
                                nc.scalar.activation(
                                    out=pt[:, j, s * 512:(s + 1) * 512],
                                    in_=tps,
                                    func=mybir.ActivationFunctionType.Exp,
                                    scale=SCALE, bias=nshift_t,
                                )
                            else:
                                # Schraudolph fast exp on the vector engine:
                                # bf16 bits via int16 view, then fp8 convert
                                sch = schp.tile([128, 512], BF16, tag="sch",
                                                name=f"sch{b}_{h}_{mc}")
                                nc.vector.tensor_scalar(
                                    out=sch.bitcast(I16),
                                    in0=tps,
                                    scalar1=EXPA, scalar2=EXPB,
                                    op0=mybir.AluOpType.mult,
                                    op1=mybir.AluOpType.add,
                                )
                                nc.vector.tensor_copy(
                                    out=pt[:, j, s * 512:(s + 1) * 512],
                                    in_=sch,
                                )

                    def emit_av(p):
                        pt = pt_pairs[p]
                        for s in range(2):
                            _mm(nc, o_ps[s],
                                vt_t[:, 2 * p:2 * p + 2,
                                     h * HD:(h + 1) * HD],
                                pt[:, :, s * 512:(s + 1) * 512],
                                start=(p == 0), stop=(p == 3),
                                perf_mode=DR)

                    emit_t(0)
                    emit_t(1)
                    emit_t(2)
                    emit_t(3)
                    emit_av(0)
                    emit_t(4)
                    emit_t(5)
                    emit_av(1)
                    emit_t(6)
                    emit_t(7)
                    emit_av(2)
                    emit_av(3)
                    # all denominator matmuls back-to-back at head end: the
                    # all-ones stationary operand loads once per head instead
                    # of once per pair (DR ldweights can't use FWL)
                    for p in range(4):
                        for s in range(2):
                            _mm(nc, s_ps[s], ones8,
                                pt_pairs[p][:, :, s * 512:(s + 1) * 512],
                                start=(p == 0), stop=(p == 3),
                                perf_mode=DR)

                    # sums are replicated across partitions: reciprocal and
                    # multiply straight out of PSUM, no broadcast needed
                    rb_sb = rbp.tile([128, N], F32, tag="rb",
                                     name=f"rb{b}_{h}")
                    for s in range(2):
                        nc.vector.reciprocal_approx_fast(
                            out=rb_sb[:, s * 512:(s + 1) * 512], in_=s_ps[s]
                        )
                        nc.vector.tensor_mul(
                            ocat[:, h, s * 512:(s + 1) * 512], o_ps[s],
                            rb_sb[:, s * 512:(s + 1) * 512],
                        )
                return ocat

            def proj(b, x_t, ocat):
                for oc in range(CC):
                    accs = [ps_work.tile([128, 512], F32, tag="w",
                                         name=f"pacc{b}_{oc}_{s}")
                            for s in range(2)]
                    for pr in range(2):
                        for s in range(2):  # consecutive mms share lhsT
                            _mm(nc, accs[s],
                                wp_sb[pr][:, :, oc * 128:(oc + 1) * 128],
                                ocat[:, 2 * pr:2 * pr + 2,
                                     s * 512:(s + 1) * 512],
                                start=(pr == 0), stop=(pr == 1),
                                perf_mode=DR)
                    for s in range(2):
                        ty = yp.tile([128, 512], F32, tag="ty",
                                     name=f"ty{b}_{oc}_{s}")
                        nc.scalar.activation(
                            out=ty, in_=accs[s],
                            func=mybir.ActivationFunctionType.Identity,
                            bias=pb_sb[:, oc:oc + 1], scale=1.0,
                        )
                        y = yp.tile([128, 512], F32, tag="y",
                                    name=f"y{b}_{oc}_{s}")
                        nc.vector.tensor_add(
                            y, ty, x_t[:, oc, s * 512:(s + 1) * 512]
                        )
                        nc.gpsimd.dma_start(
                            out=out_d[b, oc * 128:(oc + 1) * 128,
                                      s * 512:(s + 1) * 512],
                            in_=y,
                        )

            def body():
                # both batches' x-loads + norm chains kick off up front;
                # batch-1 qkv/attn fills engine gaps left by batch-0
                x0, xn0 = norm(0)
                x1, xn1 = norm(1)
                q0, k0, v0 = qkv(0, xn0)
                oc0 = attn(0, q0, k0, v0)
                q1, k1, v1 = qkv(1, xn1)
                proj(0, x0, oc0)
                oc1 = attn(1, q1, k1, v1)
                proj(1, x1, oc1)

            if reps == 1:
                body()
            elif reps < 0:  # python-unrolled, for steady-state sim analysis
                for _ in range(-reps):
                    body()
            else:
                with tc.For_i(0, reps, 1):
                    body()

    nc.compile()
    return nc


_CACHE = {}


def _get_nc():
    if "nc" not in _CACHE:
        _CACHE["nc"] = build()
    return _CACHE["nc"]


def _gmasks():
    gm = np.zeros((CC, 128, GROUPS), np.float32)
    for cc in range(CC):
        for p in range(128):
            gm[cc, p, (cc * 128 + p) // GS] = 1.0
    gmT = np.ascontiguousarray(gm.transpose(0, 2, 1))
    return gm, gmT


def _dr_pack(wT):
    """[C, cols] -> DoubleRow pair layout [2, 128, 2, cols] in fp8."""
    cols = wT.shape[1]
    return np.ascontiguousarray(
        wT.reshape(2, 2, 128, cols).transpose(0, 2, 1, 3).astype(FP8_NP))


def _prep_shared(norm_w, norm_b, qkv_w, qkv_b, proj_w, proj_b):
    """Replicated (non-batch) inputs, cast/transposed for the kernel."""
    gm_np, gmT_np = _gmasks()
    qkv_b = np.asarray(qkv_b, np.float32)
    proj_w = np.asarray(proj_w, np.float32)
    # attention(V + b_v) = attention(V) + b_v, so W_p @ b_v folds into proj_b
    pb_eff = np.asarray(proj_b, np.float32) + proj_w @ qkv_b[2 * C:]
    return {
        "norm_w": np.ascontiguousarray(np.asarray(norm_w, np.float32)),
        "norm_b": np.ascontiguousarray(np.asarray(norm_b, np.float32)),
        "qkv_w8": _dr_pack(np.asarray(qkv_w, np.float32).T),
        "qkv_b": np.ascontiguousarray(qkv_b[:2 * C]),
        "proj_w8": _dr_pack(proj_w.T),
        "proj_b": np.ascontiguousarray(pb_eff),
        "gmask": gm_np,
        "gmaskT": gmT_np,
    }


def kernel(x, norm_w, norm_b, qkv_w, qkv_b, proj_w, proj_b):
    nc = _get_nc()
    x = np.asarray(x, dtype=np.float32).reshape(B, C, N)
    shared = _prep_shared(norm_w, norm_b, qkv_w, qkv_b, proj_w, proj_b)
    in_maps = []
    for c in range(N_CORES):
        m = {"x": np.ascontiguousarray(x[c * BL:(c + 1) * BL])}
        m.update(shared)
        in_maps.append(m)
    res = run_bass_kernel_spmd(nc, in_maps, core_ids=list(range(N_CORES)))
    out = np.concatenate([res.results[c]["out"] for c in range(N_CORES)],
                         axis=0)
    return out.reshape(B, C, 32, 32).astype(np.float32)



# revision 1
# speedup vs baseline: 1.1484x; 1.1484x over previous
"""Trainium2 Bass kernel for the AttentionBlock problem.

Problem (hardcoded): x (16, 512, 32, 32) fp32; GroupNorm(32 groups) ->
1x1-conv QKV (1536x512) -> 4-head attention over 1024 tokens, head dim 128
-> 1x1-conv proj (512x512) -> residual add.

Sharding: data-parallel over batch, 2 batches per core on 8 cores; params
replicated. Weights are pre-transposed (and cast to bf16) on the host so
every matmul operand is consumed in its natural [contract-dim-on-partitions,
free] layout.

Per-core dataflow (per batch):
  - GroupNorm: per-channel bn_stats/bn_aggr in fp32, group (16-channel)
    reduction and broadcast-back via tiny fp32r matmuls against constant
    group-membership masks (host inputs).
  - QKV in bf16: q, k produced as [d, n] (heads = 128-row chunks); v
    produced directly transposed as [n, d] by swapping the matmul operands.
  - Attention per head (bf16 matmuls, fp32 PSUM accumulation):
    T = K^T Q (keys on partitions); P^T = exp(T/sqrt(hd)) — the s=0 query
    half on the scalar engine (true exp), the s=1 half on the vector engine
    via the Schraudolph bit trick (scale+shift into the bf16 exponent field,
    written through an int16 bitcast view).  No max subtraction needed:
    |T/sqrt(hd)| < ~7.  O^T accumulated over key chunks; softmax
    denominators via matmuls with an all-ones [128,128] stationary matrix,
    which lands the sums in PSUM replicated across partitions: reciprocal +
    multiply finish softmax with no broadcast step.  Each query column uses
    one exp method for all its keys, so per-query normalization stays exact.
  - Proj (bf16) + residual (fp32), streamed back to DRAM.
  - The two batches' phases are interleaved (batch-1 norm+qkv emitted
    before batch-0 proj) to hide the attention tail.
"""

import math

import numpy as np
import ml_dtypes

import concourse.mybir as mybir
import concourse.tile as tile
from concourse import bacc
from concourse.bass_utils import run_bass_kernel_spmd

# Problem constants
B, C, N = 16, 512, 1024          # batch, channels, tokens (32*32)
HEADS, HD = 4, 128               # heads, head dim
GROUPS, GS = 32, 16              # norm groups, channels per group
EPS = 1e-5
N_CORES = 8
BL = B // N_CORES                # batches per core
CC = C // 128                    # channel chunks of 128
SCALE = 1.0 / math.sqrt(HD)

F32 = mybir.dt.float32
F32R = mybir.dt.float32r
BF16 = mybir.dt.bfloat16
FP8 = mybir.dt.float8e4
I16 = mybir.dt.int16
BF16_NP = ml_dtypes.bfloat16
FP8_NP = ml_dtypes.float8_e4m3

# Softmax exp shift: P = exp(t - EXPSHIFT) keeps P in fp8e4 range (TRN max
# 240); the shift cancels exactly in the normalization.
EXPSHIFT = 4.0
# Schraudolph fast-exp into bf16 bits: bits16 = round(t*128/ln2 + b) where
# t = SCALE*s - EXPSHIFT.  b = 127*128 - 7.4 centers the mantissa-interp
# error.
EXPA = SCALE * (128.0 / math.log(2.0))
EXPB = 127.0 * 128.0 - 7.4 - EXPSHIFT * (128.0 / math.log(2.0))

DR = mybir.MatmulPerfMode.DoubleRow


def _mm(nc, out, lhsT, rhs, start=True, stop=True, perf_mode=None):
    nc.tensor.matmul(out, lhsT, rhs, start=start, stop=stop,
                     perf_mode=perf_mode)


def build(reps=1):
    nc = bacc.Bacc("TRN2", target_bir_lowering=False, debug=False)

    x_d = nc.dram_tensor("x", [BL, C, N], F32, kind="ExternalInput").ap()
    nw_d = nc.dram_tensor("norm_w", [C], F32, kind="ExternalInput").ap()
    nb_d = nc.dram_tensor("norm_b", [C], F32, kind="ExternalInput").ap()
    # DoubleRow pair layout: [pair, partition, j, cols] with contraction
    # channel c = (2*pair + j)*128 + partition
    wq_d = nc.dram_tensor("qkv_w8", [2, 128, 2, 3 * C], FP8,
                          kind="ExternalInput").ap()
    qb_d = nc.dram_tensor("qkv_b", [2 * C], F32, kind="ExternalInput").ap()
    wp_d = nc.dram_tensor("proj_w8", [2, 128, 2, C], FP8,
                          kind="ExternalInput").ap()
    pb_d = nc.dram_tensor("proj_b", [C], F32, kind="ExternalInput").ap()
    gm_d = nc.dram_tensor("gmask", [CC, 128, GROUPS], F32R,
                          kind="ExternalInput").ap()
    gmT_d = nc.dram_tensor("gmaskT", [CC, GROUPS, 128], F32R,
                           kind="ExternalInput").ap()
    out_d = nc.dram_tensor("out", [BL, C, N], F32, kind="ExternalOutput").ap()

    with tile.TileContext(nc) as tc:
        with (
            nc.allow_low_precision(reason="bf16 tiles feeding bf16 matmuls"),
            tc.tile_pool(name="const", bufs=1) as const,
            tc.tile_pool(name="xp", bufs=2) as xp,
            tc.tile_pool(name="xnp", bufs=2) as xnp,
            tc.tile_pool(name="qkp", bufs=1) as qkp,
            tc.tile_pool(name="vtp", bufs=1) as vtp,
            tc.tile_pool(name="ptp", bufs=6) as ptp,
            tc.tile_pool(name="schp", bufs=2) as schp,
            tc.tile_pool(name="xpbp", bufs=2) as xpbp,
            tc.tile_pool(name="ocp", bufs=1) as ocp,
            tc.tile_pool(name="smallp", bufs=4) as smallp,
            tc.tile_pool(name="rbp", bufs=2) as rbp,
            tc.tile_pool(name="yp", bufs=4) as yp,
            tc.tile_pool(name="ps_work", bufs=4, space="PSUM") as ps_work,
            tc.tile_pool(name="ps_o", bufs=2, space="PSUM") as ps_o,
            tc.tile_pool(name="ps_sum", bufs=2, space="PSUM") as ps_sum,
        ):
            # ---- constants / weights (loaded once) ----
            # ones first: it feeds the PE warm-up matmuls and the softmax
            # denominator (DoubleRow) matmuls
            ones8 = const.tile([128, 2, 128], FP8, name="ones8")
            nc.vector.memset(ones8, 1.0)
            for wi in range(2):
                wu_ps = ps_work.tile([128, 128], F32, tag="w",
                                     name=f"wu{wi}")
                for wj in range(6):
                    _mm(nc, wu_ps, ones8[:, 0, :], ones8[:, 0, :],
                        start=(wj == 0), stop=(wj == 5))

            # small consts + masks next: they gate the first stats matmuls
            w_sb = const.tile([128, CC], F32, name="w_sb")
            nc.scalar.dma_start(
                out=w_sb, in_=nw_d.rearrange("(cc p) -> p cc", p=128))
            b_sb = const.tile([128, CC], F32, name="b_sb")
            nc.scalar.dma_start(
                out=b_sb, in_=nb_d.rearrange("(cc p) -> p cc", p=128))
            qb_sb = const.tile([128, 8], F32, name="qb_sb")
            nc.scalar.dma_start(
                out=qb_sb, in_=qb_d.rearrange("(oc p) -> p oc", p=128)
            )
            pb_sb = const.tile([128, CC], F32, name="pb_sb")
            nc.scalar.dma_start(
                out=pb_sb, in_=pb_d.rearrange("(cc p) -> p cc", p=128))

            eps_t = const.tile([GROUPS, 1], F32, name="eps_t")
            nc.vector.memset(eps_t, EPS)
            nshift_t = const.tile([128, 1], F32, name="nshift_t")
            nc.vector.memset(nshift_t, -EXPSHIFT)

            gm = []
            gmT = []
            for cc in range(CC):
                t = const.tile([128, GROUPS], F32R, name=f"gm{cc}")
                nc.scalar.dma_start(out=t, in_=gm_d[cc])
                tT = const.tile([GROUPS, 128], F32R, name=f"gmT{cc}")
                nc.scalar.dma_start(out=tT, in_=gmT_d[cc])
                gm.append(t)
                gmT.append(tT)

            wq_sb = []
            for pr in range(2):
                t = const.tile([128, 2, 3 * C], FP8, name=f"wq{pr}")
                nc.sync.dma_start(out=t, in_=wq_d[pr])
                wq_sb.append(t)
            wp_sb = []
            for pr in range(2):
                t = const.tile([128, 2, C], FP8, name=f"wp{pr}")
                nc.scalar.dma_start(out=t, in_=wp_d[pr])
                wp_sb.append(t)

            # ---- per batch pipeline (phases interleaved across batches) ----
            def norm(b):
                x_t = xp.tile([128, CC, N], F32, tag="x", name=f"x_t{b}")
                xr = x_d[b].rearrange("(cc p) n -> p cc n", p=128)
                for cc in range(CC):
                    eng = nc.gpsimd if cc % 2 == 0 else nc.scalar
                    eng.dma_start(out=x_t[:, cc, :], in_=xr[:, cc, :])

                # group stats via per-channel bn_stats/bn_aggr
                cols = smallp.tile([128, CC, 2], F32R, tag="mv",
                                   name=f"cols{b}")
                for cc in range(CC):
                    stats = smallp.tile([128, 2, 6], F32, tag="stats",
                                        name=f"stats{b}_{cc}")
                    for s in range(2):
                        nc.vector.bn_stats(
                            out=stats[:, s, :],
                            in_=x_t[:, cc, s * 512:(s + 1) * 512],
                        )
                    mv_f = smallp.tile([128, 2], F32, tag="mvf",
                                       name=f"mvf{b}_{cc}")
                    nc.vector.bn_aggr(out=mv_f, in_=stats)
                    # mv[:,1] := E[x^2] = var + mean^2
                    msq = smallp.tile([128, 1], F32, tag="msq",
                                      name=f"msq{b}_{cc}")
                    nc.vector.tensor_mul(msq, mv_f[:, 0:1], mv_f[:, 0:1])
                    nc.vector.tensor_add(mv_f[:, 1:2], mv_f[:, 1:2], msq)
                    nc.vector.tensor_copy(out=cols[:, cc, :], in_=mv_f)

                gstats = ps_sum.tile([GROUPS, 2], F32, tag="s",
                                     name=f"gstats{b}")
                for cc in range(CC):
                    _mm(nc, gstats, gm[cc], cols[:, cc, :],
                        start=(cc == 0), stop=(cc == CC - 1))
                grp = smallp.tile([GROUPS, 2], F32R, tag="grp", name=f"grp{b}")
                nc.scalar.mul(out=grp, in_=gstats, mul=1.0 / GS)
                gvar = smallp.tile([GROUPS, 1], F32, tag="gvar",
                                   name=f"gvar{b}")
                nc.vector.tensor_mul(gvar, grp[:, 0:1], grp[:, 0:1])
                nc.vector.tensor_sub(gvar, grp[:, 1:2], gvar)
                nc.vector.tensor_scalar(
                    out=gvar, in0=gvar, scalar1=EPS, scalar2=None,
                    op0=mybir.AluOpType.add,
                )
                # rstd via bit-trick rsqrt + 2 Newton steps, all on DVE:
                # avoids the ACT Sqrt<->Exp table switch every iteration
                y0 = smallp.tile([GROUPS, 1], F32, tag="y0", name=f"y0{b}")
                nc.vector.tensor_scalar(
                    out=y0.bitcast(mybir.dt.int32),
                    in0=gvar.bitcast(mybir.dt.int32),
                    scalar1=1, scalar2=-1,
                    op0=mybir.AluOpType.logical_shift_right,
                    op1=mybir.AluOpType.bitwise_xor,
                )
                nc.vector.tensor_scalar(
                    out=y0.bitcast(mybir.dt.int32),
                    in0=y0.bitcast(mybir.dt.int32),
                    scalar1=0x5f3759df + 1, scalar2=None,
                    op0=mybir.AluOpType.add,
                )
                hv = smallp.tile([GROUPS, 1], F32, tag="hv", name=f"hv{b}")
                nc.vector.tensor_scalar(
                    out=hv, in0=gvar, scalar1=-0.5, scalar2=None,
                    op0=mybir.AluOpType.mult,
                )
                for it in range(2):
                    yy = smallp.tile([GROUPS, 1], F32, tag="yy",
                                     name=f"yy{b}_{it}")
                    nc.vector.tensor_mul(yy, y0, y0)
                    nc.vector.tensor_mul(yy, yy, hv)
                    nc.vector.tensor_scalar(
                        out=yy, in0=yy, scalar1=1.5, scalar2=None,
                        op0=mybir.AluOpType.add,
                    )
                    nc.vector.tensor_mul(y0, y0, yy)
                nc.vector.tensor_copy(out=grp[:, 1:2], in_=y0)

                # broadcast per-group (mean, rstd) back to channels, all
                # four chunk matmuls into one PSUM tile, then one vectorized
                # scale/bias computation for every chunk at once
                xn_t = xnp.tile([128, CC, N], FP8, tag="xn", name=f"xn{b}")
                bc = ps_sum.tile([128, CC, 2], F32, tag="s",
                                 name=f"bc{b}")
                for cc in range(CC):
                    _mm(nc, bc[:, cc, :], gmT[cc], grp)
                ab = smallp.tile([128, CC, 2], F32, tag="ab", name=f"ab{b}")
                # a = rstd*w  (strided [:,:,1] lanes)
                nc.vector.tensor_mul(ab[:, :, 1], bc[:, :, 1], w_sb)
                # b = norm_b - mean*a
                nc.vector.tensor_mul(ab[:, :, 0], bc[:, :, 0], ab[:, :, 1])
                nc.vector.tensor_sub(ab[:, :, 0], b_sb, ab[:, :, 0])
                for cc in range(CC):
                    if cc % 2 == 0:
                        nc.vector.tensor_scalar(
                            out=xn_t[:, cc, :], in0=x_t[:, cc, :],
                            scalar1=ab[:, cc, 1:2], scalar2=ab[:, cc, 0:1],
                            op0=mybir.AluOpType.mult, op1=mybir.AluOpType.add,
                        )
                    else:
                        nc.scalar.activation(
                            out=xn_t[:, cc, :], in_=x_t[:, cc, :],
                            func=mybir.ActivationFunctionType.Identity,
                            bias=ab[:, cc, 0:1], scale=ab[:, cc, 1:2],
                        )
                return x_t, xn_t

            def qkv(b, xn_t):
                # qkv via fp8 DoubleRow (contraction pairs of 128-ch chunks)
                q_t = qkp.tile([128, HEADS, N], BF16, tag="q", name=f"q{b}")
                k_t = qkp.tile([128, HEADS, N], BF16, tag="k", name=f"k{b}")
                vt_t = vtp.tile([128, 8, C], FP8, tag="vt", name=f"vt{b}")

                def v_chunk(tc_i):
                    acc = ps_work.tile([128, 512], F32, tag="w",
                                       name=f"vacc{b}_{tc_i}")
                    for pr in range(2):
                        _mm(nc, acc,
                            xn_t[:, 2 * pr:2 * pr + 2,
                                 tc_i * 128:(tc_i + 1) * 128],
                            wq_sb[pr][:, :, 2 * C:3 * C],
                            start=(pr == 0), stop=(pr == 1),
                            perf_mode=DR)
                    nc.vector.tensor_copy(out=vt_t[:, tc_i, :], in_=acc)

                # head h's q (oc=h) and k (oc=4+h) emitted adjacently so
                # attention of head 0 starts after two oc blocks, with the
                # v chunks it needs interleaved right behind
                for oc in [0, 4, 1, 5, 2, 6, 3, 7]:
                    dst = q_t if oc < 4 else k_t
                    h = oc % 4
                    accs = [ps_work.tile([128, 512], F32, tag="w",
                                         name=f"qkacc{b}_{oc}_{s}")
                            for s in range(2)]
                    for pr in range(2):
                        for s in range(2):  # consecutive mms share lhsT
                            _mm(nc, accs[s],
                                wq_sb[pr][:, :, oc * 128:(oc + 1) * 128],
                                xn_t[:, 2 * pr:2 * pr + 2,
                                     s * 512:(s + 1) * 512],
                                start=(pr == 0), stop=(pr == 1),
                                perf_mode=DR)
                    for s in range(2):  # both bias-adds on ACT: DVE is the
                        nc.scalar.activation(  # busier engine here
                            out=dst[:, h, s * 512:(s + 1) * 512],
                            in_=accs[s],
                            func=mybir.ActivationFunctionType.Identity,
                            bias=qb_sb[:, oc:oc + 1], scale=1.0,
                        )
                # v bias is folded into proj_b on the host
                # (attention(V + b) = attention(V) + b), so vt is a plain
                # PSUM evacuation
                vt_t = vtp.tile([128, 8, C], FP8, tag="vt", name=f"vt{b}")
                for tc_i in range(8):
                    acc = ps_work.tile([128, 512], F32, tag="w",
                                       name=f"vacc{b}_{tc_i}")
                    for pr in range(2):
                        _mm(nc, acc,
                            xn_t[:, 2 * pr:2 * pr + 2,
                                 tc_i * 128:(tc_i + 1) * 128],
                            wq_sb[pr][:, :, 2 * C:3 * C],
                            start=(pr == 0), stop=(pr == 1),
                            perf_mode=DR)
                    nc.vector.tensor_copy(out=vt_t[:, tc_i, :], in_=acc)

                return q_t, k_t, vt_t

            def attn(b, q_t, k_t, vt_t):
                ocat = ocp.tile([128, HEADS, N], FP8, tag="ocat",
                                name=f"ocat{b}")
                for h in range(HEADS):
                    o_ps = [ps_o.tile([128, 512], F32, tag="o",
                                      name=f"o{b}_{h}_{i}") for i in range(2)]
                    s_ps = [ps_sum.tile([128, 512], F32, tag="s",
                                        name=f"s{b}_{h}_{i}")
                            for i in range(2)]
                    # P^T stored as fp8 key-chunk pairs for DoubleRow AV
                    pt_pairs = [
                        ptp.tile([128, 2, N], FP8, tag="pt",
                                 name=f"pt{b}_{h}_{p}")
                        for p in range(4)
                    ]

                    def emit_t(mc):
                        pt = pt_pairs[mc // 2]
                        j = mc % 2
                        for s in range(2):
                            tps = ps_work.tile([128, 512], F32, tag="w",
                                               name=f"t{b}_{h}_{mc}_{s}")
                            _mm(nc, tps,
                                k_t[:, h, mc * 128:(mc + 1) * 128],
                                q_t[:, h, s * 512:(s + 1) * 512])
                            if (mc * 2 + s) % 16 < 11:
                                # true exp on the scalar engine (fp8 out)
                                nc.scalar.activation(
                                    out=pt[:, j, s * 512:(s + 1) * 512],
                                    in_=tps,
                                    func=mybir.ActivationFunctionType.Exp,
                                    scale=SCALE, bias=nshift_t,
                                )
                            else:
                                # Schraudolph fast exp on the vector engine:
                                # bf16 bits via int16 view, then fp8 convert
                                sch = schp.tile([128, 512], BF16, tag="sch",
                                                name=f"sch{b}_{h}_{mc}")
                                nc.vector.tensor_scalar(
                                    out=sch.bitcast(I16),
                                    in0=tps,
                                    scalar1=EXPA, scalar2=EXPB,
                                    op0=mybir.AluOpType.mult,
                                    op1=mybir.AluOpType.add,
                                )
                                nc.vector.tensor_copy(
                                    out=pt[:, j, s * 512:(s + 1) * 512],
                                    in_=sch,
                                )

                    def emit_av(p):
                        pt = pt_pairs[p]
                        for s in range(2):
                            _mm(nc, o_ps[s],
                                vt_t[:, 2 * p:2 * p + 2,
                                     h * HD:(h + 1) * HD],
                                pt[:, :, s * 512:(s + 1) * 512],
                                start=(p == 0), stop=(p == 3),
                                perf_mode=DR)

                    emit_t(0)
                    emit_t(1)
                    emit_t(2)
                    emit_t(3)
                    emit_av(0)
                    emit_t(4)
                    emit_t(5)
                    emit_av(1)
                    emit_t(6)
                    emit_t(7)
                    emit_av(2)
                    emit_av(3)
                    # all denominator matmuls back-to-back at head end: the
                    # all-ones stationary operand loads once per head instead
                    # of once per pair (DR ldweights can't use FWL)
                    for p in range(4):
                        for s in range(2):
                            _mm(nc, s_ps[s], ones8,
                                pt_pairs[p][:, :, s * 512:(s + 1) * 512],
                                start=(p == 0), stop=(p == 3),
                                perf_mode=DR)

                    # sums are replicated across partitions: reciprocal and
                    # multiply straight out of PSUM, no broadcast needed
                    rb_sb = rbp.tile([128, N], F32, tag="rb",
                                     name=f"rb{b}_{h}")
                    for s in range(2):
                        nc.vector.reciprocal_approx_fast(
                            out=rb_sb[:, s * 512:(s + 1) * 512], in_=s_ps[s]
                        )
                        nc.vector.tensor_mul(
                            ocat[:, h, s * 512:(s + 1) * 512], o_ps[s],
                            rb_sb[:, s * 512:(s + 1) * 512],
                        )
                return ocat

            def proj(b, x_t, ocat):
                for oc in range(CC):
                    accs = [ps_work.tile([128, 512], F32, tag="w",
                                         name=f"pacc{b}_{oc}_{s}")
                            for s in range(2)]
                    for pr in range(2):
                        for s in range(2):  # consecutive mms share lhsT
                            _mm(nc, accs[s],
                                wp_sb[pr][:, :, oc * 128:(oc + 1) * 128],
                                ocat[:, 2 * pr:2 * pr + 2,
                                     s * 512:(s + 1) * 512],
                                start=(pr == 0), stop=(pr == 1),
                                perf_mode=DR)
                    for s in range(2):
                        ty = yp.tile([128, 512], F32, tag="ty",
                                     name=f"ty{b}_{oc}_{s}")
                        nc.scalar.activation(
                            out=ty, in_=accs[s],
                            func=mybir.ActivationFunctionType.Identity,
                            bias=pb_sb[:, oc:oc + 1], scale=1.0,
                        )
                        y = yp.tile([128, 512], F32, tag="y",
                                    name=f"y{b}_{oc}_{s}")
                        nc.vector.tensor_add(
                            y, ty, x_t[:, oc, s * 512:(s + 1) * 512]
                        )
                        nc.gpsimd.dma_start(
                            out=out_d[b, oc * 128:(oc + 1) * 128,
                                      s * 512:(s + 1) * 512],
                            in_=y,
                        )

            def body():
                # both batches' x-loads + norm chains kick off up front;
                # batch-1 qkv/attn fills engine gaps left by batch-0
                x0, xn0 = norm(0)
                x1, xn1 = norm(1)
                q0, k0, v0 = qkv(0, xn0)
                oc0 = attn(0, q0, k0, v0)
                q1, k1, v1 = qkv(1, xn1)
                proj(0, x0, oc0)
                oc1 = attn(1, q1, k1, v1)
                proj(1, x1, oc1)

            if reps == 1:
                body()
            elif reps < 0:  # python-unrolled, for steady-state sim analysis
                for _ in range(-reps):
                    body()
            else:
                with tc.For_i(0, reps, 1):
                    body()

    nc.compile()
    return nc


_CACHE = {}


def _get_nc():
    if "nc" not in _CACHE:
        _CACHE["nc"] = build()
    return _CACHE["nc"]


def _gmasks():
    gm = np.zeros((CC, 128, GROUPS), np.float32)
    for cc in range(CC):
        for p in range(128):
            gm[cc, p, (cc * 128 + p) // GS] = 1.0
    gmT = np.ascontiguousarray(gm.transpose(0, 2, 1))
    return gm, gmT


def _dr_pack(wT):
    """[C, cols] -> DoubleRow pair layout [2, 128, 2, cols] in fp8."""
    cols = wT.shape[1]
    return np.ascontiguousarray(
        wT.reshape(2, 2, 128, cols).transpose(0, 2, 1, 3).astype(FP8_NP))


def _prep_shared(norm_w, norm_b, qkv_w, qkv_b, proj_w, proj_b):
    """Replicated (non-batch) inputs, cast/transposed for the kernel."""
    gm_np, gmT_np = _gmasks()
    qkv_b = np.asarray(qkv_b, np.float32)
    proj_w = np.asarray(proj_w, np.float32)
    # attention(V + b_v) = attention(V) + b_v, so W_p @ b_v folds into proj_b
    pb_eff = np.asarray(proj_b, np.float32) + proj_w @ qkv_b[2 * C:]
    return {
        "norm_w": np.ascontiguousarray(np.asarray(norm_w, np.float32)),
        "norm_b": np.ascontiguousarray(np.asarray(norm_b, np.float32)),
        "qkv_w8": _dr_pack(np.asarray(qkv_w, np.float32).T),
        "qkv_b": np.ascontiguousarray(qkv_b[:2 * C]),
        "proj_w8": _dr_pack(proj_w.T),
        "proj_b": np.ascontiguousarray(pb_eff),
        "gmask": gm_np,
        "gmaskT": gmT_np,
    }


def kernel(x, norm_w, norm_b, qkv_w, qkv_b, proj_w, proj_b):
    nc = _get_nc()
    x = np.asarray(x, dtype=np.float32).reshape(B, C, N)
    shared = _prep_shared(norm_w, norm_b, qkv_w, qkv_b, proj_w, proj_b)
    in_maps = []
    for c in range(N_CORES):
        m = {"x": np.ascontiguousarray(x[c * BL:(c + 1) * BL])}
        m.update(shared)
        in_maps.append(m)
    res = run_bass_kernel_spmd(nc, in_maps, core_ids=list(range(N_CORES)))
    out = np.concatenate([res.results[c]["out"] for c in range(N_CORES)],
                         axis=0)
    return out.reshape(B, C, 32, 32).astype(np.float32)

